# revision 12
# baseline (speedup 1.0000x reference)
"""Trainium2 Bass kernel for BERTSpanNER boundary scores (v2).

out[b,i,j,l] = min(cum[j+1,l]-cum[i,l], -EPS, begin[i,l], end[j,l]) for j>=i,
else -1e9, where cum/begin/end derive from log_softmax(x @ W + b) per label's
I,B,L,U tag group.

Sharding: 8 cores = 4 batches x 2 label-halves (8 labels each), SPMD.

v2 design:
- Transposed prologue: W-stationary bf16 matmul gives logits^T [tag, seq];
  tag-group sums and log-softmax differences via two small selector matmuls;
  per-label cumsum rows via tensor_tensor_scan; C/G per-partition via PE
  transposes.
- Far-field shortcut: for j >= i0+192 every span is >=66 tokens long, so
  has_no_hole <= -120 << min(G, E2) >= -4.9 and the output is exactly
  bf16(A[j]-C[i]) - a single subtract (Scalar activation or 1-op DVE ts),
  no min ops. Near region (192 cols) does sub+minG per label plus ONE fused
  3D-AP tensor_tensor min with E2 per row tile.
- Device writes only j >= i0 in l-major (S, LC, S) bf16; host fills the
  constant -1e9 lower triangle (including the in-tile j<i part) and
  transposes to [i, j, l] f32.
"""
import os
import sys

for _p in ("/opt/trn_rl_repo", "/root/.axon_site/_ro/trn_rl_repo"):
    if os.path.isdir(_p) and _p not in sys.path:
        sys.path.insert(0, _p)

import numpy as np
import concourse.bacc as bacc
import concourse.mybir as mybir
from concourse.bass import _add_dep_helper
from concourse.tile import TileContext
from concourse.bass_utils import run_bass_kernel_spmd
from concourse.alu_op_type import AluOpType

F32 = mybir.dt.float32
BF16 = mybir.dt.bfloat16
AF = mybir.ActivationFunctionType

B, S, H, NL = 4, 1024, 400, 16
NT = 1 + 4 * NL          # 65
EPS = 1e-8
NEG = -1e9
P = 128
NST = S // P             # 8 row tiles
LC = NL // 2             # 8 labels per core
KT = [128, 128, 128, 17]  # k-tiling of H+1=401
NEARL = 192              # cols [i0, i0+NEARL) get the full 3-way min
NSC = 6                  # labels 0..NSC-1 use the Scalar-subtract near path
GS_SC = 0.40             # gpsimd share of far cols (Scalar labels)
GS_DV = 0.50             # gpsimd share of far cols (DVE labels)

_CACHED_NC = None


def _build():
    nc = bacc.Bacc()
    xk = nc.declare_dram_parameter("xk", [P, 4 * S], BF16, isOutput=False)
    Wk = nc.declare_dram_parameter("Wk", [P, 4 * NT], BF16, isOutput=False)
    selc = nc.declare_dram_parameter("selc", [P, 32], BF16, isOutput=False)
    sel2c = nc.declare_dram_parameter("sel2c", [P, 96], F32, isOutput=False)
    eye = nc.declare_dram_parameter("eye", [P, P], F32, isOutput=False)
    out = nc.declare_dram_parameter("out", [S, LC * S], BF16, isOutput=True)

    a_row_d = nc.dram_tensor("a_row_d", [LC, S + 1], F32)
    e2_row_d = nc.dram_tensor("e2_row_d", [LC, S], BF16)

    with TileContext(nc) as tc:
        with tc.tile_pool(name="const", bufs=1) as cpool, \
             tc.tile_pool(name="work", bufs=1) as wpool, \
             tc.tile_pool(name="u", bufs=2) as upool, \
             tc.tile_pool(name="oc", bufs=3) as opool, \
             tc.tile_pool(name="ps_mm", bufs=1, space="PSUM") as psmm, \
             tc.tile_pool(name="ps_tr", bufs=2, space="PSUM") as pstr:

            # scalar engine: force Exp act-table load before data arrives
            dm = cpool.tile([1, 1], F32, tag="dm")
            nc.vector.memset(dm[:], 0.0)
            dmo = cpool.tile([1, 1], F32, tag="dmo")
            nc.scalar.activation(dmo[:], dm[:], AF.Exp)

            # ---------------- input loads (small weights first) --------------
            wk_sb = cpool.tile([P, 4 * NT], BF16, tag="wk_sb")
            nc.sync.dma_start(out=wk_sb[:], in_=Wk[:])
            selc_sb = cpool.tile([P, 32], BF16, tag="selc_sb")
            nc.scalar.dma_start(out=selc_sb[:], in_=selc[:])
            sel2c_sb = cpool.tile([P, 96], F32, tag="sel2c_sb")
            nc.scalar.dma_start(out=sel2c_sb[:], in_=sel2c[:])
            eye_sb = cpool.tile([P, P], F32, tag="eye_sb")
            nc.gpsimd.dma_start(out=eye_sb[:], in_=eye[:])
            xk_sb = cpool.tile([P, 4 * S], BF16, tag="xk_sb")
            ring3 = [nc.sync, nc.scalar, nc.gpsimd]
            for c in range(2):
                for ki in range(4):
                    eng = ring3[(c * 4 + ki) % 3]
                    sl = slice(ki * S + c * 512, ki * S + c * 512 + 512)
                    eng.dma_start(out=xk_sb[:, sl], in_=xk[:, sl])

            # ---------------- logits^T = (x@W+b)^T  [tag, seq] ---------------
            pl = [psmm.tile([P, 512], F32, name="pl%d" % c, tag="pl%d" % c)
                  for c in range(2)]
            for ki, kt in enumerate(KT):
                for c in range(2):
                    nc.tensor.matmul(
                        pl[c][:NT, :],
                        wk_sb[0:kt, ki * NT:(ki + 1) * NT],
                        xk_sb[0:kt, ki * S + c * 512: ki * S + c * 512 + 512],
                        start=ki == 0, stop=ki == 3)

            # logits are tiny (|x@W| < ~4), exp needs no max-stabilization
            expT = wpool.tile([NT, S], BF16, tag="expT")
            for c in range(2):
                nc.scalar.activation(expT[:, c * 512:(c + 1) * 512],
                                     pl[c][:NT, :], AF.Exp)

            # ---------------- tag-group sums [25, seq] -----------------------
            ps25 = [psmm.tile([P, 512], F32, name="ps25_%d" % c, tag="ps25_%d" % c)
                    for c in range(2)]
            for c in range(2):
                nc.tensor.matmul(ps25[c][:32, :], selc_sb[0:NT, :],
                                 expT[:, c * 512:(c + 1) * 512],
                                 start=True, stop=True)
            lnsb = wpool.tile([32, S], F32, tag="lnsb")
            for c in range(2):
                nc.scalar.activation(lnsb[:25, c * 512:(c + 1) * 512],
                                     ps25[c][:25, :], AF.Ln)

            # rows: inside at partitions 0-7, G at 32-39, lend at 64-71
            # (PSUM reads must start at a 32-aligned partition)
            ps24 = [psmm.tile([P, 512], F32, name="ps24_%d" % c, tag="ps24_%d" % c)
                    for c in range(2)]
            for c in range(2):
                nc.tensor.matmul(ps24[c][:96, :], sel2c_sb[0:25, :],
                                 lnsb[:25, c * 512:(c + 1) * 512],
                                 start=True, stop=True)

            # ---------------- derived rows -----------------------------------
            gsb = wpool.tile([LC, S], F32, tag="gsb")       # G rows (for PE)
            e2sb = wpool.tile([LC, S], BF16, tag="e2sb")    # E2 rows (bf16)
            for c in range(2):
                cs = slice(c * 512, (c + 1) * 512)
                nc.vector.tensor_copy(e2sb[:, cs], ps24[c][64:72, :])
                nc.vector.tensor_copy(gsb[:, cs], ps24[c][32:40, :])

            # E2 broadcast first: ready earlier than A, and the fused tt
            # needs all labels
            E2_b = wpool.tile([P, LC * S], BF16, tag="e2_b")
            dma_w_e = nc.sync.dma_start(out=e2_row_d[:], in_=e2sb[:])
            for l in range(LC):
                re = (nc.sync if l % 2 == 0 else nc.scalar).dma_start(
                    out=E2_b[:, l * S:(l + 1) * S],
                    in_=e2_row_d[l:l + 1, :].rearrange(
                        "o f -> (o f)").partition_broadcast(P))
                _add_dep_helper(re.ins, dma_w_e.ins, True, "e2 bcast RAW")
            E2_b3 = E2_b[:].rearrange("p (l j) -> p l j", l=LC)

            # A rows: cumsum of inside along seq, with leading zero column
            asb = wpool.tile([LC, S + 1], F32, tag="asb")
            nc.vector.memset(asb[:, 0:1], 0.0)
            nc.vector.tensor_tensor_scan(asb[:, 1:513], ps24[0][0:LC, :],
                                         gsb[:, 0:512], 0.0,
                                         AluOpType.add, AluOpType.bypass)
            nc.vector.tensor_tensor_scan(asb[:, 513:1025], ps24[1][0:LC, :],
                                         gsb[:, 512:1024], asb[:, 512:513],
                                         AluOpType.add, AluOpType.bypass)

            # per-label A broadcast tiles: sweep of label l starts as soon as
            # its own broadcast lands
            dma_w_a = nc.sync.dma_start(out=a_row_d[:], in_=asb[:])
            A_bl = []
            for l in range(LC):
                ab = wpool.tile([P, S], F32, name="a_b%d" % l, tag="a_b%d" % l)
                ra = (nc.sync if l % 2 == 0 else nc.scalar).dma_start(
                    out=ab[:],
                    in_=a_row_d[l:l + 1, 1:S + 1].rearrange(
                        "o f -> (o f)").partition_broadcast(P))
                _add_dep_helper(ra.ins, dma_w_a.ins, True, "a bcast RAW")
                A_bl.append(ab)

            # ---------------- C, G' per-partition via PE transposes ----------
            ncs64 = wpool.tile([P, NST * LC], F32, tag="ncs64")   # -C
            g64 = wpool.tile([P, NST * LC], F32, tag="g64")       # min(G,-EPS)
            for t in range(NST):
                trc = pstr.tile([P, 512], F32, tag="ps_tr")
                nc.tensor.transpose(trc[:P, 0:LC], asb[:, t * P: t * P + P],
                                    eye_sb[0:LC, 0:LC])
                nc.vector.tensor_scalar(ncs64[:, t * LC:(t + 1) * LC],
                                        trc[:, 0:LC], -1.0, None,
                                        AluOpType.mult)
                trg = pstr.tile([P, 512], F32, tag="ps_tr")
                nc.tensor.transpose(trg[:P, 0:LC],
                                    gsb[:, t * P: t * P + P],
                                    eye_sb[0:LC, 0:LC])
                nc.vector.tensor_scalar(g64[:, t * LC:(t + 1) * LC],
                                        trg[:, 0:LC], -EPS, None,
                                        AluOpType.min)

            # ---------------- main sweep -------------------------------------
            # Near [0,nw): sub + minG per label, then one fused 3D minE2 per t.
            # Far [nw,W): plain A-C subtract, split Scalar/DVE/GpSimd.
            out3 = out[:].rearrange("(t p) f -> t p f", p=P)
            for t in range(NST):
                i0 = t * P
                W = S - i0
                nw = min(NEARL, W)
                farW = W - nw
                oc = opool.tile([P, LC * W], BF16, tag="oc")
                oc3 = oc[:].rearrange("p (l j) -> p l j", j=W)
                u_t = upool.tile([P, LC * nw], BF16, tag="u_t")
                u3 = u_t[:].rearrange("p (l j) -> p l j", j=nw)
                for l in range(LC):
                    A_b = A_bl[l]
                    ncs_s = ncs64[:, t * LC + l: t * LC + l + 1]
                    g_s = g64[:, t * LC + l: t * LC + l + 1]
                    if l < NSC:
                        # gpsimd far chunk (from the right)
                        gw = (int(farW * GS_SC) // 64) * 64
                        if gw < 128:
                            gw = 0
                        fsS = W - gw
                        # one Scalar op covers near + its far share
                        nc.scalar.activation(oc3[:, l, 0:fsS],
                                             A_b[:, i0:i0 + fsS],
                                             AF.Identity, bias=ncs_s)
                        nc.vector.tensor_scalar(u3[:, l, :], oc3[:, l, 0:nw],
                                                g_s, None, AluOpType.min)
                        if gw > 0:
                            nc.gpsimd.tensor_scalar(
                                oc3[:, l, fsS:W], A_b[:, i0 + fsS:i0 + W],
                                ncs_s, None, AluOpType.add)
                    else:
                        nc.vector.tensor_scalar(
                            u3[:, l, :], A_b[:, i0:i0 + nw],
                            ncs_s, g_s, AluOpType.add, AluOpType.min)
                        if farW > 0:
                            gw = (int(farW * GS_DV) // 64) * 64
                            if gw < 128:
                                gw = 0
                            fsD = W - gw
                            nc.vector.tensor_scalar(
                                oc3[:, l, nw:fsD], A_b[:, i0 + nw:i0 + fsD],
                                ncs_s, None, AluOpType.add)
                            if gw > 0:
                                nc.gpsimd.tensor_scalar(
                                    oc3[:, l, fsD:W], A_b[:, i0 + fsD:i0 + W],
                                    ncs_s, None, AluOpType.add)
                # one fused min-with-E2 across all labels for this row tile
                nc.vector.tensor_tensor(oc3[:, :, 0:nw], u3[:],
                                        E2_b3[:, :, i0:i0 + nw], AluOpType.min)
                dst = out3[t, :, :].rearrange("p (l j) -> p l j", l=LC)[:, :, i0:S]
                (nc.sync if t % 2 == 0 else nc.scalar).dma_start(out=dst, in_=oc3)

    nc.compile()
    return nc


def _bf16(a):
    u = np.ascontiguousarray(a, dtype=np.float32).view(np.uint32)
    r = ((u >> 16) & 1) + 0x7FFF
    return ((u + r) >> 16).astype(np.uint16)


def _unbf16(a):
    return (a.astype(np.uint32) << 16).view(np.float32)


def _host_inputs(x, W, b):
    """Per-core inputs. Core c: batch c//2, label half c%2."""
    x = np.asarray(x, dtype=np.float32)
    W = np.asarray(W, dtype=np.float32)
    b = np.asarray(b, dtype=np.float32)

    Wb = np.concatenate([W, b[None, :]], axis=0)          # (401, 65)
    wkp = np.zeros((4 * P, NT), np.float32)
    wkp[:H + 1] = Wb
    wk = _bf16(wkp.reshape(4, P, NT).transpose(1, 0, 2).reshape(P, 4 * NT))
    eye = np.eye(P, dtype=np.float32)
    sel2 = np.zeros((P, 96), np.float32)
    cols = np.concatenate([np.arange(8), 32 + np.arange(8), 64 + np.arange(8)])
    sel2[0, cols] = -1.0
    sel2[1 + np.arange(24), cols] = 1.0

    in_maps = []
    for c in range(8):
        bb, h = c // 2, c % 2
        xTb = np.concatenate([x[bb].T, np.ones((1, S), np.float32)], axis=0)
        xp = np.zeros((4 * P, S), np.float32)
        xp[:H + 1] = xTb
        xkc = _bf16(xp.reshape(4, P, S).transpose(1, 0, 2).reshape(P, 4 * S))
        sel = np.zeros((P, 32), np.float32)
        sel[:NT, 0] = 1.0
        for g in range(LC):
            lg = h * LC + g
            base = 1 + 4 * lg
            sel[base:base + 4, 1 + g] = 1.0          # I,B,L,U
            sel[[base + 1, base + 3], 9 + g] = 1.0   # B,U -> begin
            sel[[base + 2, base + 3], 17 + g] = 1.0  # L,U -> end
        in_maps.append({
            "xk": xkc, "Wk": wk, "selc": _bf16(sel), "sel2c": sel2,
            "eye": eye,
        })
    return in_maps


def kernel(x, mask, W, b, _collect=None):
    global _CACHED_NC
    if _CACHED_NC is None:
        _CACHED_NC = _build()
    nc = _CACHED_NC
    in_maps = _host_inputs(x, W, b)
    res = run_bass_kernel_spmd(nc, in_maps, list(range(8)))
    if _collect is not None:
        _collect.append(res)
    outf = np.empty((B, S, S, NL), dtype=np.float32)
    for c in range(8):
        bb, h = c // 2, c % 2
        o = res.results[c]["out"]
        if o.dtype != np.uint16:
            o = o.view(np.uint16)
        o = _unbf16(o).reshape(S, LC, S)              # [i, l, j]
        outf[bb, :, :, h * LC:(h + 1) * LC] = o.transpose(0, 2, 1)
    # constant lower triangle (j < i) filled on host
    for i in range(1, S):
        outf[:, i, :i, :] = NEG
    return outf


# revision 13
# speedup vs baseline: 2.1203x; 2.1203x over previous
"""Trainium2 Bass kernel for BERTSpanNER boundary scores (v2).

out[b,i,j,l] = min(cum[j+1,l]-cum[i,l], -EPS, begin[i,l], end[j,l]) for j>=i,
else -1e9, where cum/begin/end derive from log_softmax(x @ W + b) per label's
I,B,L,U tag group.

Sharding: 8 cores = 4 batches x 2 label-halves (8 labels each), SPMD.

v2 design:
- Transposed prologue: W-stationary bf16 matmul gives logits^T [tag, seq];
  tag-group sums and log-softmax differences via two small selector matmuls;
  per-label cumsum rows via tensor_tensor_scan; C/G per-partition via PE
  transposes.
- Far-field shortcut: for j >= i0+192 every span is >=66 tokens long, so
  has_no_hole <= -120 << min(G, E2) >= -4.9 and the output is exactly
  bf16(A[j]-C[i]) - a single subtract (Scalar activation or 1-op DVE ts),
  no min ops. Near region (192 cols) does sub+minG per label plus ONE fused
  3D-AP tensor_tensor min with E2 per row tile.
- Device writes only j >= i0 in l-major (S, LC, S) bf16; host fills the
  constant -1e9 lower triangle (including the in-tile j<i part) and
  transposes to [i, j, l] f32.
"""
import os
import sys

for _p in ("/opt/trn_rl_repo", "/root/.axon_site/_ro/trn_rl_repo"):
    if os.path.isdir(_p) and _p not in sys.path:
        sys.path.insert(0, _p)

import numpy as np
import concourse.bacc as bacc
import concourse.mybir as mybir
from concourse.bass import _add_dep_helper
from concourse.tile import TileContext
from concourse.bass_utils import run_bass_kernel_spmd
from concourse.alu_op_type import AluOpType

F32 = mybir.dt.float32
BF16 = mybir.dt.bfloat16
AF = mybir.ActivationFunctionType

B, S, H, NL = 4, 1024, 400, 16
NT = 1 + 4 * NL          # 65
EPS = 1e-8
NEG = -1e9
P = 128
NST = S // P             # 8 row tiles
LC = NL // 2             # 8 labels per core
KT = [128, 128, 128, 17]  # k-tiling of H+1=401
NEARL = 192              # cols [i0, i0+NEARL) get the full 3-way min
NSC = 7                  # labels 0..NSC-1 use the Scalar-subtract path
FD_SC = 0.50             # DVE share of far cols for Scalar labels

_CACHED_NC = None


def _build():
    nc = bacc.Bacc()
    xk = nc.declare_dram_parameter("xk", [P, 4 * S], BF16, isOutput=False)
    Wk = nc.declare_dram_parameter("Wk", [P, 4 * NT], BF16, isOutput=False)
    selc = nc.declare_dram_parameter("selc", [P, 32], BF16, isOutput=False)
    sel2c = nc.declare_dram_parameter("sel2c", [P, 96], F32, isOutput=False)
    eye = nc.declare_dram_parameter("eye", [P, P], F32, isOutput=False)
    out = nc.declare_dram_parameter("out", [S, LC * S], BF16, isOutput=True)

    a_row_d = nc.dram_tensor("a_row_d", [LC, S + 1], F32)
    e2_row_d = nc.dram_tensor("e2_row_d", [LC, S], BF16)

    with TileContext(nc) as tc:
        with tc.tile_pool(name="const", bufs=1) as cpool, \
             tc.tile_pool(name="work", bufs=1) as wpool, \
             tc.tile_pool(name="oc", bufs=3) as opool, \
             tc.tile_pool(name="ps_mm", bufs=1, space="PSUM") as psmm, \
             tc.tile_pool(name="ps_tr", bufs=2, space="PSUM") as pstr:

            # scalar engine: force Exp act-table load before data arrives
            dm = cpool.tile([1, 1], F32, tag="dm")
            nc.vector.memset(dm[:], 0.0)
            dmo = cpool.tile([1, 1], F32, tag="dmo")
            nc.scalar.activation(dmo[:], dm[:], AF.Exp)

            # ---------------- input loads (small weights first) --------------
            wk_sb = cpool.tile([P, 4 * NT], BF16, tag="wk_sb")
            nc.sync.dma_start(out=wk_sb[:], in_=Wk[:])
            selc_sb = cpool.tile([P, 32], BF16, tag="selc_sb")
            nc.scalar.dma_start(out=selc_sb[:], in_=selc[:])
            sel2c_sb = cpool.tile([P, 96], F32, tag="sel2c_sb")
            nc.scalar.dma_start(out=sel2c_sb[:], in_=sel2c[:])
            eye_sb = cpool.tile([P, P], F32, tag="eye_sb")
            nc.gpsimd.dma_start(out=eye_sb[:], in_=eye[:])
            xk_sb = cpool.tile([P, 4 * S], BF16, tag="xk_sb")
            ring3 = [nc.sync, nc.scalar, nc.gpsimd]
            for c in range(2):
                for ki in range(4):
                    eng = ring3[(c * 4 + ki) % 3]
                    sl = slice(ki * S + c * 512, ki * S + c * 512 + 512)
                    eng.dma_start(out=xk_sb[:, sl], in_=xk[:, sl])

            # ---------------- logits^T = (x@W+b)^T  [tag, seq] ---------------
            pl = [psmm.tile([P, 512], F32, name="pl%d" % c, tag="pl%d" % c)
                  for c in range(2)]
            for ki, kt in enumerate(KT):
                for c in range(2):
                    nc.tensor.matmul(
                        pl[c][:NT, :],
                        wk_sb[0:kt, ki * NT:(ki + 1) * NT],
                        xk_sb[0:kt, ki * S + c * 512: ki * S + c * 512 + 512],
                        start=ki == 0, stop=ki == 3)

            # logits are tiny (|x@W| < ~4), exp needs no max-stabilization
            expT = wpool.tile([NT, S], BF16, tag="expT")
            for c in range(2):
                nc.scalar.activation(expT[:, c * 512:(c + 1) * 512],
                                     pl[c][:NT, :], AF.Exp)

            # ---------------- tag-group sums [25, seq] -----------------------
            ps25 = [psmm.tile([P, 512], F32, name="ps25_%d" % c, tag="ps25_%d" % c)
                    for c in range(2)]
            for c in range(2):
                nc.tensor.matmul(ps25[c][:32, :], selc_sb[0:NT, :],
                                 expT[:, c * 512:(c + 1) * 512],
                                 start=True, stop=True)
            lnsb = wpool.tile([32, S], F32, tag="lnsb")
            for c in range(2):
                nc.scalar.activation(lnsb[:25, c * 512:(c + 1) * 512],
                                     ps25[c][:25, :], AF.Ln)

            # rows: inside at partitions 0-7, G at 32-39, lend at 64-71
            # (PSUM reads must start at a 32-aligned partition)
            ps24 = [psmm.tile([P, 512], F32, name="ps24_%d" % c, tag="ps24_%d" % c)
                    for c in range(2)]
            for c in range(2):
                nc.tensor.matmul(ps24[c][:96, :], sel2c_sb[0:25, :],
                                 lnsb[:25, c * 512:(c + 1) * 512],
                                 start=True, stop=True)

            # ---------------- derived rows -----------------------------------
            gsb = wpool.tile([LC, S], F32, tag="gsb")       # G rows (for PE)
            e2sb = wpool.tile([LC, S], BF16, tag="e2sb")    # E2 rows (bf16)
            for c in range(2):
                cs = slice(c * 512, (c + 1) * 512)
                nc.vector.tensor_copy(e2sb[:, cs], ps24[c][64:72, :])
                nc.vector.tensor_copy(gsb[:, cs], ps24[c][32:40, :])

            # E2 broadcast first: ready earlier than A, and the fused tt
            # needs all labels
            E2_b = wpool.tile([P, LC * S], BF16, tag="e2_b")
            dma_w_e = nc.sync.dma_start(out=e2_row_d[:], in_=e2sb[:])
            for l in range(LC):
                re = (nc.sync if l % 2 == 0 else nc.scalar).dma_start(
                    out=E2_b[:, l * S:(l + 1) * S],
                    in_=e2_row_d[l:l + 1, :].rearrange(
                        "o f -> (o f)").partition_broadcast(P))
                _add_dep_helper(re.ins, dma_w_e.ins, True, "e2 bcast RAW")
            E2_b3 = E2_b[:].rearrange("p (l j) -> p l j", l=LC)

            # A rows: cumsum of inside along seq, with leading zero column
            asb = wpool.tile([LC, S + 1], F32, tag="asb")
            nc.vector.memset(asb[:, 0:1], 0.0)
            nc.vector.tensor_tensor_scan(asb[:, 1:513], ps24[0][0:LC, :],
                                         gsb[:, 0:512], 0.0,
                                         AluOpType.add, AluOpType.bypass)
            nc.vector.tensor_tensor_scan(asb[:, 513:1025], ps24[1][0:LC, :],
                                         gsb[:, 512:1024], asb[:, 512:513],
                                         AluOpType.add, AluOpType.bypass)

            # per-label A broadcast tiles: sweep of label l starts as soon as
            # its own broadcast lands
            dma_w_a = nc.sync.dma_start(out=a_row_d[:], in_=asb[:])
            A_bl = []
            for l in range(LC):
                ab = wpool.tile([P, S], F32, name="a_b%d" % l, tag="a_b%d" % l)
                ra = (nc.sync if l % 2 == 0 else nc.scalar).dma_start(
                    out=ab[:],
                    in_=a_row_d[l:l + 1, 1:S + 1].rearrange(
                        "o f -> (o f)").partition_broadcast(P))
                _add_dep_helper(ra.ins, dma_w_a.ins, True, "a bcast RAW")
                A_bl.append(ab)

            # ---------------- C, G' per-partition via PE transposes ----------
            ncs64 = wpool.tile([P, NST * LC], F32, tag="ncs64")   # -C
            g64 = wpool.tile([P, NST * LC], F32, tag="g64")       # min(G,-EPS)
            for t in range(NST):
                trc = pstr.tile([P, 512], F32, tag="ps_tr")
                nc.tensor.transpose(trc[:P, 0:LC], asb[:, t * P: t * P + P],
                                    eye_sb[0:LC, 0:LC])
                nc.vector.tensor_scalar(ncs64[:, t * LC:(t + 1) * LC],
                                        trc[:, 0:LC], -1.0, None,
                                        AluOpType.mult)
                trg = pstr.tile([P, 512], F32, tag="ps_tr")
                nc.tensor.transpose(trg[:P, 0:LC],
                                    gsb[:, t * P: t * P + P],
                                    eye_sb[0:LC, 0:LC])
                nc.vector.tensor_scalar(g64[:, t * LC:(t + 1) * LC],
                                        trg[:, 0:LC], -EPS, None,
                                        AluOpType.min)

            # ---------------- main sweep -------------------------------------
            # Near [0,nw): sub + minG per label, then one fused 3D minE2 per t.
            # Far [nw,W): plain A-C subtract (minG folded in where free), split
            # Scalar/DVE. All ops write oc in place.
            out3 = out[:].rearrange("(t p) f -> t p f", p=P)
            for t in range(NST):
                i0 = t * P
                W = S - i0
                nw = min(NEARL, W)
                farW = W - nw
                oc = opool.tile([P, LC * W], BF16, tag="oc")
                oc3 = oc[:].rearrange("p (l j) -> p l j", j=W)
                for l in range(LC):
                    A_b = A_bl[l]
                    ncs_s = ncs64[:, t * LC + l: t * LC + l + 1]
                    g_s = g64[:, t * LC + l: t * LC + l + 1]
                    if l < NSC:
                        fd = (int(farW * FD_SC) // 64) * 64
                        if fd < 64:
                            fd = 0
                        fsS = W - fd
                        # one Scalar op: A-C over near + its far share
                        nc.scalar.activation(oc3[:, l, 0:fsS],
                                             A_b[:, i0:i0 + fsS],
                                             AF.Identity, bias=ncs_s)
                        # in-place min with G' on the near part only
                        nc.vector.tensor_scalar(oc3[:, l, 0:nw],
                                                oc3[:, l, 0:nw],
                                                g_s, None, AluOpType.min)
                        if fd > 0:
                            nc.vector.tensor_scalar(
                                oc3[:, l, fsS:W], A_b[:, i0 + fsS:i0 + W],
                                ncs_s, None, AluOpType.add)
                    else:
                        # one DVE op over the whole width: (A-C) min G'
                        # (the min is a no-op beyond the near region)
                        nc.vector.tensor_scalar(
                            oc3[:, l, 0:W], A_b[:, i0:i0 + W],
                            ncs_s, g_s, AluOpType.add, AluOpType.min)
                # fused in-place min-with-E2 across all labels, near cols
                nc.vector.tensor_tensor(oc3[:, :, 0:nw], oc3[:, :, 0:nw],
                                        E2_b3[:, :, i0:i0 + nw], AluOpType.min)
                dst = out3[t, :, :].rearrange("p (l j) -> p l j", l=LC)[:, :, i0:S]
                (nc.sync if t % 2 == 0 else nc.scalar).dma_start(out=dst, in_=oc3)

    nc.compile()
    return nc


def _bf16(a):
    u = np.ascontiguousarray(a, dtype=np.float32).view(np.uint32)
    r = ((u >> 16) & 1) + 0x7FFF
    return ((u + r) >> 16).astype(np.uint16)


def _unbf16(a):
    return (a.astype(np.uint32) << 16).view(np.float32)


def _host_inputs(x, W, b):
    """Per-core inputs. Core c: batch c//2, label half c%2."""
    x = np.asarray(x, dtype=np.float32)
    W = np.asarray(W, dtype=np.float32)
    b = np.asarray(b, dtype=np.float32)

    Wb = np.concatenate([W, b[None, :]], axis=0)          # (401, 65)
    wkp = np.zeros((4 * P, NT), np.float32)
    wkp[:H + 1] = Wb
    wk = _bf16(wkp.reshape(4, P, NT).transpose(1, 0, 2).reshape(P, 4 * NT))
    eye = np.eye(P, dtype=np.float32)
    sel2 = np.zeros((P, 96), np.float32)
    cols = np.concatenate([np.arange(8), 32 + np.arange(8), 64 + np.arange(8)])
    sel2[0, cols] = -1.0
    sel2[1 + np.arange(24), cols] = 1.0

    in_maps = []
    for c in range(8):
        bb, h = c // 2, c % 2
        xTb = np.concatenate([x[bb].T, np.ones((1, S), np.float32)], axis=0)
        xp = np.zeros((4 * P, S), np.float32)
        xp[:H + 1] = xTb
        xkc = _bf16(xp.reshape(4, P, S).transpose(1, 0, 2).reshape(P, 4 * S))
        sel = np.zeros((P, 32), np.float32)
        sel[:NT, 0] = 1.0
        for g in range(LC):
            lg = h * LC + g
            base = 1 + 4 * lg
            sel[base:base + 4, 1 + g] = 1.0          # I,B,L,U
            sel[[base + 1, base + 3], 9 + g] = 1.0   # B,U -> begin
            sel[[base + 2, base + 3], 17 + g] = 1.0  # L,U -> end
        in_maps.append({
            "xk": xkc, "Wk": wk, "selc": _bf16(sel), "sel2c": sel2,
            "eye": eye,
        })
    return in_maps


def kernel(x, mask, W, b, _collect=None):
    global _CACHED_NC
    if _CACHED_NC is None:
        _CACHED_NC = _build()
    nc = _CACHED_NC
    in_maps = _host_inputs(x, W, b)
    res = run_bass_kernel_spmd(nc, in_maps, list(range(8)))
    if _collect is not None:
        _collect.append(res)
    outf = np.empty((B, S, S, NL), dtype=np.float32)
    for c in range(8):
        bb, h = c // 2, c % 2
        o = res.results[c]["out"]
        if o.dtype != np.uint16:
            o = o.view(np.uint16)
        o = _unbf16(o).reshape(S, LC, S)              # [i, l, j]
        outf[bb, :, :, h * LC:(h + 1) * LC] = o.transpose(0, 2, 1)
    # constant lower triangle (j < i) filled on host
    for i in range(1, S):
        outf[:, i, :i, :] = NEG
    return outf
